# revision 19
# baseline (speedup 1.0000x reference)
"""Trainium2 Bass kernel for nn_ALiBi_76278619176977 (sparse_attention).

reference:
    S = (q @ k^T) * F**-0.5                       # [b,q,k]
    W = softmax(S, axis=-1)                       # unmasked softmax
    dist = sqrt(sum((cq - ck)^2, -1))             # pairwise coord distance
    msd = where(alibi, 0, dist/running_mean*bias_scale)
    out = where(attn, 0, W - msd) @ v             # [b,q,f]

Strategy: data-parallel over b=16 -> 2 batches per core, 8 cores, no
cross-core communication.  Per core, q-partition layout, per 128-row q-tile:

  PSUM_s = qT.T @ kT  (fp16 matmuls, N=512)
  E, s   = ACT Exp(SCALE * PSUM_s) with accum_out  # unnormalized softmax + sums
  PSUM_d = Aq.T @ Ak  (fp16 K_c=4 augmented matmul)
           Aq rows [1, 1, xq, yq];  Ak rows [nk_hi, nk_lo, -2xk, -2yk]
           where nk_hi+nk_lo is a double-fp16 split of |ck|^2, all coords
           fp16-rounded -> PSUM_d = |ck|^2 - 2cq.ck exactly (fp32 accum)
  zs     = ACT Sqrt(u2 * PSUM_d + u2*(nq+eps))  # scale/bias per-partition APs
           = s*c*dist        (nq = |cq|^2 in fp32 bias => d^2 >= -1e-6,
                              eps=1e-5; c = bias_scale/running_mean, u2=(s*c)^2)
  P      = (E - zs*aNOT) * tNOT    # fp16 DVE tensor_tensor x3 (2x mode);
                                   # aNOT/tNOT host-negated masks as fp16
  PT     = dma-xbar transpose of P (16x [128,128] fp16)
  att    = sum_c PT_c.T @ v_c (PSUM fp32);  out = att * (1/s)

= (1/s)*[tNOT*E @ v] - [tNOT*aNOT*c*dist @ v]  == reference.

q-tiles are processed in groups of PH=8: all exps of a group first, then all
sqrts (exp and sqrt live in different ACT table sets; interleaving would pay
~2.7us per switch).
"""

import sys

sys.path.insert(0, "/opt/trn_rl_repo")

import numpy as np

import concourse.bacc as bacc
import concourse.bass as bass
import concourse.tile as tile
from concourse.tile import add_dep_helper
from concourse import mybir

DT = mybir.dt
AF = mybir.ActivationFunctionType
OP = mybir.AluOpType

# problem shape (hardcoded per spec)
B, Q, K, F, C = 16, 2048, 2048, 128, 2
NCORES = 8
BL = B // NCORES  # batches per core
P = 128
QT = Q // P  # q tiles per batch
KC = K // P  # k chunks (for PV matmul)
NB = 512  # one PSUM bank of fp32
HALF = 1024  # psum tile free size (2 banks)
PH = 8  # q-tiles per exp/sqrt phase group
SCALE = float(F) ** -0.5
EPS_D2 = 1e-5  # fp32 rounding guard so sqrt input never goes negative

_CACHE = {}


def build_program(ablate=None):
    nc = bacc.Bacc("TRN2", target_bir_lowering=False, debug=False)

    qT_d = nc.dram_tensor("qT", [BL, F, Q], DT.float32, kind="ExternalInput")
    kT_d = nc.dram_tensor("kT", [BL, F, K], DT.float32, kind="ExternalInput")
    # v pre-chunked on host: v_d[b, p, t, f] = v[b, t*128+p, f]
    v_d = nc.dram_tensor("v", [BL, P, KC, F], DT.float32, kind="ExternalInput")
    # coords transposed + flattened: [x row | y row] on one partition
    cqT_d = nc.dram_tensor("cqT", [BL, 1, C * Q], DT.float32, kind="ExternalInput")
    ckT_d = nc.dram_tensor("ckT", [BL, 1, C * K], DT.float32, kind="ExternalInput")
    cqn_d = nc.dram_tensor("cqn", [BL, Q, C], DT.float32, kind="ExternalInput")
    tn_d = nc.dram_tensor("tn", [BL, Q, K], DT.uint8, kind="ExternalInput")
    an_d = nc.dram_tensor("an", [BL, Q, K], DT.uint8, kind="ExternalInput")
    bs_d = nc.dram_tensor("bs", [P, 1], DT.float32, kind="ExternalInput")
    rm_d = nc.dram_tensor("rm", [P, 1], DT.float32, kind="ExternalInput")
    ones16_d = nc.dram_tensor("ones16", [1, Q], DT.float16, kind="ExternalInput")
    out_d = nc.dram_tensor("out", [BL, Q, F], DT.float32, kind="ExternalOutput")

    with tile.TileContext(nc) as tc:
        with (
            tc.tile_pool(name="consts", bufs=1) as consts,
            tc.tile_pool(name="stage", bufs=2) as stage,
            tc.tile_pool(name="binp", bufs=2) as binp,  # per-batch fp16 inputs
            tc.tile_pool(name="augp", bufs=1) as augp,
            tc.tile_pool(name="rowp", bufs=2) as rowp,
            tc.tile_pool(name="Ep", bufs=5) as Ep,
            tc.tile_pool(name="maskp", bufs=5) as maskp,
            tc.tile_pool(name="zsp", bufs=2) as zsp,
            tc.tile_pool(name="lnp", bufs=2) as lnp,
            tc.tile_pool(name="ptp", bufs=4) as ptp,
            tc.tile_pool(name="smallp", bufs=8) as smallp,
            tc.tile_pool(name="outp", bufs=3) as outp,
            tc.tile_pool(name="psp", bufs=3, space="PSUM") as psp,
            tc.tile_pool(name="attps", bufs=2, space="PSUM") as attps,
        ):
            # ---------- per-core constants
            bs_sb = consts.tile([P, 1], DT.float32)
            nc.sync.dma_start(bs_sb, bs_d[:])
            rm_sb = consts.tile([P, 1], DT.float32)
            nc.sync.dma_start(rm_sb, rm_d[:])
            zeros16 = consts.tile([P, K], DT.float16)
            nc.vector.memset(zeros16, 0.0)
            c2_sb = consts.tile([P, 1], DT.float32)
            # c2 = (bs / rm)^2
            nc.vector.reciprocal(c2_sb, rm_sb[:])
            nc.vector.tensor_tensor(c2_sb, c2_sb[:], bs_sb[:], OP.mult)
            nc.vector.tensor_tensor(c2_sb, c2_sb[:], c2_sb[:], OP.mult)

            # PV work queue, drained a few chunk-matmuls at a time between
            # other PE work so xbar-transpose latency never stalls the PE
            # stream and PV never bunches up.
            pv_q = []  # dicts: b, t, pt_sb, r_sb, att_ps, next_c, age
            v_sbs = {}
            slot_serial = [0]

            def drain_pv(budget, after=None, min_age=2):
                first = True
                while budget > 0 and pv_q:
                    job = pv_q[0]
                    if slot_serial[0] - job["age"] < min_age and min_age > 0:
                        break
                    if job["att_ps"] is None:
                        job["att_ps"] = attps.tile([P, F], DT.float32, tag="att", name="att_ps")
                    c = job["next_c"]
                    mm = nc.tensor.matmul(
                        job["att_ps"][:],
                        job["pt_sb"][:, c * P : (c + 1) * P],
                        v_sbs[job["b"]][:, c, :],
                        start=(c == 0),
                        stop=(c == KC - 1),
                    )
                    if first and after is not None:
                        add_dep_helper(
                            mm.ins, after.ins, sync=False,
                            reason="PE order: PV chunks after dist/S matmuls",
                        )
                        first = False
                    job["next_c"] += 1
                    budget -= 1
                    if job["next_c"] == KC:
                        o_sb = outp.tile([P, F], DT.float32, tag="o")
                        nc.vector.tensor_scalar(
                            o_sb, job["att_ps"][:], job["r_sb"][:], None, OP.mult
                        )
                        nc.sync.dma_start(
                            out_d[job["b"], job["t"] * P : (job["t"] + 1) * P, :], o_sb
                        )
                        pv_q.pop(0)

            for b in range(BL):
                # ---------- per-batch setup
                qT_st = stage.tile([P, Q], DT.float32, tag="qkstage")
                nc.gpsimd.dma_start(qT_st, qT_d[b])
                qT_sb = binp.tile([P, Q], DT.float16, tag="qTb")
                nc.vector.tensor_copy(qT_sb, qT_st[:])

                kT_st = stage.tile([P, K], DT.float32, tag="qkstage")
                nc.gpsimd.dma_start(kT_st, kT_d[b])
                kT_sb = binp.tile([P, K], DT.float16, tag="kTb")
                nc.vector.tensor_copy(kT_sb, kT_st[:])

                v_st = stage.tile([P, KC, F], DT.float32, tag="qkstage")
                nc.gpsimd.dma_start(v_st, v_d[b])
                v_sb = binp.tile([P, KC, F], DT.float16, tag="vb")
                nc.vector.tensor_copy(v_sb, v_st[:])
                v_sbs[b] = v_sb

                # per-q |cq|^2 column from fp16-rounded coords (fp32 result)
                cqn_st = stage.tile([P, QT, C], DT.float32, tag="cqn_st")
                nc.gpsimd.dma_start(cqn_st, cqn_d[b].rearrange("(t p) c -> p t c", p=P))
                cqn16 = stage.tile([P, QT, C], DT.float16, tag="cqn16")
                nc.vector.tensor_copy(cqn16, cqn_st[:])
                sqn = stage.tile([P, QT, C], DT.float32, tag="sqn")
                nc.vector.tensor_tensor(sqn, cqn16[:], cqn16[:], OP.mult)
                nq_col = binp.tile([P, QT], DT.float32, tag="nq_col")
                nc.vector.tensor_reduce(nq_col, sqn[:], axis=mybir.AxisListType.X, op=OP.add)

                # ---------- augmented distance matrices (all fp16-exact)
                # Aq rows: [1, 1, xq, yq]   Ak rows: [nk_hi, nk_lo, -2xk, -2yk]
                Aq_sb = augp.tile([4, Q], DT.float16, tag="Aq")
                Ak_sb = augp.tile([4, K], DT.float16, tag="Ak")
                # q side: fp16 coords
                row_q = rowp.tile([1, C * Q], DT.float32, tag="scr32", bufs=1)
                nc.sync.dma_start(row_q, cqT_d[b])
                xyq16 = rowp.tile([1, C * Q], DT.float16, tag="scr16")
                nc.vector.tensor_copy(xyq16, row_q[:])
                nc.sync.dma_start(Aq_sb[0:1, :], ones16_d[:])
                nc.sync.dma_start(Aq_sb[1:2, :], ones16_d[:])
                nc.sync.dma_start(Aq_sb[2:3, :], xyq16[:, 0:Q])
                nc.sync.dma_start(Aq_sb[3:4, :], xyq16[:, Q : 2 * Q])
                # k side: -2*fp16 coords (exact doubling) + double-fp16 norms
                row_k = rowp.tile([1, C * K], DT.float32, tag="scr32", bufs=1)
                nc.sync.dma_start(row_k, ckT_d[b])
                xyk16 = rowp.tile([1, C * K], DT.float16, tag="scr16")
                nc.vector.tensor_copy(xyk16, row_k[:])
                m2k16 = rowp.tile([1, C * K], DT.float16, tag="scr16")
                nc.vector.tensor_scalar(m2k16, xyk16[:], -2.0, None, OP.mult)
                sqk = rowp.tile([1, C * K], DT.float32, tag="scr32", bufs=1)
                nc.vector.tensor_tensor(sqk, xyk16[:], xyk16[:], OP.mult)
                nk32 = rowp.tile([1, K], DT.float32, tag="nk32", bufs=1)
                nc.vector.tensor_tensor(nk32, sqk[:, 0:K], sqk[:, K : 2 * K], OP.add)
                nk_hi = rowp.tile([1, K], DT.float16, tag="scr16")
                nc.vector.tensor_copy(nk_hi, nk32[:])
                nk_lo = rowp.tile([1, K], DT.float16, tag="scr16")
                nc.vector.tensor_tensor(nk_lo, nk32[:], nk_hi[:], OP.subtract)
                nc.sync.dma_start(Ak_sb[0:1, :], nk_hi[:])
                nc.sync.dma_start(Ak_sb[1:2, :], nk_lo[:])
                nc.sync.dma_start(Ak_sb[2:3, :], m2k16[:, 0:K])
                nc.sync.dma_start(Ak_sb[3:4, :], m2k16[:, K : 2 * K])

                for t in range(QT):
                    # ---- S matmuls + softmax exp (unmasked, accum -> s)
                    E_sb = Ep.tile([P, K], DT.float16, tag="E")
                    s01 = smallp.tile([P, 2], DT.float32, tag="s01")
                    for h in range(2):
                        ps = psp.tile([P, HALF], DT.float32, tag="ps")
                        for j in range(2):
                            col = 2 * h + j
                            last_s = nc.tensor.matmul(
                                ps[:, j * NB : (j + 1) * NB],
                                qT_sb[:, t * P : (t + 1) * P],
                                kT_sb[:, col * NB : (col + 1) * NB],
                                start=True,
                                stop=True,
                            )
                        nc.scalar.activation(
                            E_sb[:, h * HALF : (h + 1) * HALF],
                            ps[:],
                            AF.Exp,
                            bias=0.0,
                            scale=SCALE,
                            accum_out=s01[:, h : h + 1],
                        )
                    s_sb = smallp.tile([P, 1], DT.float32, tag="s")
                    nc.vector.tensor_tensor(s_sb, s01[:, 0:1], s01[:, 1:2], OP.add)
                    u2_sb = smallp.tile([P, 1], DT.float32, tag="u2")
                    nc.vector.tensor_tensor(u2_sb, s_sb[:], s_sb[:], OP.mult)
                    nc.vector.tensor_tensor(u2_sb, u2_sb[:], c2_sb[:], OP.mult)
                    # ln bias = u2 * (nq + eps)
                    eb_sb = smallp.tile([P, 1], DT.float32, tag="eb")
                    nc.vector.scalar_tensor_tensor(
                        eb_sb, nq_col[:, t : t + 1], EPS_D2, u2_sb[:], OP.add, OP.mult
                    )
                    r_sb = smallp.tile([P, 1], DT.float32, tag="r")
                    nc.vector.reciprocal(r_sb, s_sb[:])

                    # ---- dist matmuls + zs = exp(0.5*ln(u2*d2 + u2*(nq+eps)))
                    lnb = lnp.tile([P, K], DT.float32, tag="lnb")
                    last_d2 = None
                    if ablate != "nodist":
                        for h in range(2):
                            psd = psp.tile([P, HALF], DT.float32, tag="ps")
                            for j in range(2):
                                col = 2 * h + j
                                last_d2 = nc.tensor.matmul(
                                    psd[:, j * NB : (j + 1) * NB],
                                    Aq_sb[:, t * P : (t + 1) * P],
                                    Ak_sb[:, col * NB : (col + 1) * NB],
                                    start=True,
                                    stop=True,
                                )
                            nc.scalar.activation(
                                lnb[:, h * HALF : (h + 1) * HALF],
                                psd[:],
                                AF.Ln,
                                bias=eb_sb[:],
                                scale=u2_sb[:],
                            )
                    zs_sb = zsp.tile([P, K], DT.float16, tag="zs")
                    if ablate != "nodist":
                        nc.scalar.activation(zs_sb, lnb[:], AF.Exp, bias=0.0, scale=0.5)

                    if ablate != "nomask":
                        an_sb = maskp.tile([P, K], DT.uint8, tag="an")
                        nc.gpsimd.dma_start(an_sb, an_d[b, t * P : (t + 1) * P, :])
                        tn_sb = maskp.tile([P, K], DT.uint8, tag="tn")
                        nc.gpsimd.dma_start(tn_sb, tn_d[b, t * P : (t + 1) * P, :])

                    # P = where(attn, 0, E - where(alibi, 0, zs))  (in place)
                    if ablate == "nomask":
                        if ablate != "nodist":
                            nc.vector.tensor_tensor(E_sb, E_sb[:], zs_sb[:], OP.subtract)
                    elif ablate == "nodist":
                        nc.vector.copy_predicated(E_sb, tn_sb[:], zeros16[:])
                    else:
                        nc.vector.copy_predicated(zs_sb, an_sb[:], zeros16[:])
                        nc.vector.tensor_tensor(E_sb, E_sb[:], zs_sb[:], OP.subtract)
                        nc.vector.copy_predicated(E_sb, tn_sb[:], zeros16[:])

                    if ablate == "nopv":
                        o_sb = outp.tile([P, F], DT.float32, tag="o")
                        nc.vector.tensor_scalar(
                            o_sb, E_sb[:, 0:F], r_sb[:], None, OP.mult
                        )
                        nc.sync.dma_start(out_d[b, t * P : (t + 1) * P, :], o_sb)
                        slot_serial[0] += 1
                        continue
                    # transpose P -> PT chunks: one xbar transpose does all
                    # 16 chunks (out[p, t, f] = P[f, t*128+p])
                    pt_sb = ptp.tile([P, K], DT.float16, tag="pt")
                    nc.sync.dma_start_transpose(
                        pt_sb[:].rearrange("p (t f) -> p t f", t=KC),
                        E_sb[:],
                    )
                    pv_q.append(dict(
                        b=b, t=t, pt_sb=pt_sb, r_sb=r_sb,
                        att_ps=None, next_c=0, age=slot_serial[0],
                    ))
                    slot_serial[0] += 1
                    drain_pv(16, after=last_d2 if last_d2 is not None else last_s)

            drain_pv(10**9, min_age=0)

    nc.compile()
    return nc


def _prep_in_maps(q, k, v, coords_q, coords_k, attn_mask, alibi_mask, bias_scale, running_mean):
    q = np.asarray(q, dtype=np.float32)
    k = np.asarray(k, dtype=np.float32)
    v = np.asarray(v, dtype=np.float32)
    coords_q = np.asarray(coords_q, dtype=np.float32)
    coords_k = np.asarray(coords_k, dtype=np.float32)
    # raw masks as u8 predicates for copy_predicated
    tn = np.asarray(attn_mask, dtype=bool).astype(np.uint8)
    an = np.asarray(alibi_mask, dtype=bool).astype(np.uint8)
    bs = np.broadcast_to(np.asarray(bias_scale, np.float32).reshape(1, 1), (P, 1)).copy()
    rm = np.broadcast_to(np.asarray(running_mean, np.float32).reshape(1, 1), (P, 1)).copy()
    ones16 = np.ones((1, Q), dtype=np.float16)

    in_maps = []
    for i in range(NCORES):
        sl = slice(i * BL, (i + 1) * BL)
        in_maps.append(
            dict(
                qT=np.ascontiguousarray(q[sl].transpose(0, 2, 1)),
                kT=np.ascontiguousarray(k[sl].transpose(0, 2, 1)),
                v=np.ascontiguousarray(
                    v[sl].reshape(BL, KC, P, F).transpose(0, 2, 1, 3)
                ),
                cqT=np.ascontiguousarray(coords_q[sl].transpose(0, 2, 1)).reshape(BL, 1, C * Q),
                ckT=np.ascontiguousarray(coords_k[sl].transpose(0, 2, 1)).reshape(BL, 1, C * K),
                cqn=np.ascontiguousarray(coords_q[sl]),
                tn=np.ascontiguousarray(tn[sl]),
                an=np.ascontiguousarray(an[sl]),
                bs=bs,
                rm=rm,
                ones16=ones16,
            )
        )
    return in_maps


def kernel(q, k, v, coords_q, coords_k, attn_mask, alibi_mask, bias_scale, running_mean):
    from concourse.bass_utils import run_bass_kernel_spmd

    if "nc" not in _CACHE:
        _CACHE["nc"] = build_program()
    nc = _CACHE["nc"]

    in_maps = _prep_in_maps(
        q, k, v, coords_q, coords_k, attn_mask, alibi_mask, bias_scale, running_mean
    )
    res = run_bass_kernel_spmd(nc, in_maps, core_ids=list(range(NCORES)))
    _CACHE["last_results"] = res
    out = np.concatenate([res.results[i]["out"] for i in range(NCORES)], axis=0)
    return out.astype(np.float32)


# revision 21
# speedup vs baseline: 1.3011x; 1.3011x over previous
"""Trainium2 Bass kernel for nn_ALiBi_76278619176977 (sparse_attention).

reference:
    S = (q @ k^T) * F**-0.5                       # [b,q,k]
    W = softmax(S, axis=-1)                       # unmasked softmax
    dist = sqrt(sum((cq - ck)^2, -1))             # pairwise coord distance
    msd = where(alibi, 0, dist/running_mean*bias_scale)
    out = where(attn, 0, W - msd) @ v             # [b,q,f]

Strategy: data-parallel over b=16 -> 2 batches per core, 8 cores, no
cross-core communication.  Per core, q-partition layout, per 128-row q-tile:

  PSUM_s = qT.T @ kT  (fp16 matmuls, N=512)
  E, s   = ACT Exp(SCALE * PSUM_s) with accum_out  # unnormalized softmax + sums
  PSUM_d = Aq.T @ Ak  (fp16 K_c=4 augmented matmul)
           Aq rows [1, 1, xq, yq];  Ak rows [nk_hi, nk_lo, -2xk, -2yk]
           where nk_hi+nk_lo is a double-fp16 split of |ck|^2, all coords
           fp16-rounded -> PSUM_d = |ck|^2 - 2cq.ck exactly (fp32 accum)
  zs     = ACT Sqrt(u2 * PSUM_d + u2*(nq+eps))  # scale/bias per-partition APs
           = s*c*dist        (nq = |cq|^2 in fp32 bias => d^2 >= -1e-6,
                              eps=1e-5; c = bias_scale/running_mean, u2=(s*c)^2)
  P      = where(attn, 0, E - where(alibi, 0, zs))   # u8 masks, DVE
           copy_predicated (raw mask bytes as predicates)
  PT     = one dma-xbar transpose of P: out[p,c,f] = P[f, c*128+p]
  att    = sum_c PT_c.T @ v_c (PSUM fp32);  out = att * (1/s)

= (1/s)*[NOT(attn)*E @ v] - [NOT(attn)*NOT(alibi)*c*dist @ v]  == reference.

Scheduling notes:
 - q-tiles processed in groups of PH=8: all exps of a group, then all sqrts
   (exp and sqrt live in different ACT table sets; interleaving would pay
   ~2.7us per switch).
 - PV matmuls are drained chunk-wise from a queue with a 2-tile minimum age
   so the xbar-transpose latency never stalls the in-order PE stream.
 - mask/input loads are dispatched from the (otherwise idle) GpSimd queue so
   waiting transposes on the Sync queue cannot block prefetches.
"""

import sys

sys.path.insert(0, "/opt/trn_rl_repo")

import numpy as np

import concourse.bacc as bacc
import concourse.bass as bass
import concourse.tile as tile
from concourse.tile import add_dep_helper
from concourse import mybir

DT = mybir.dt
AF = mybir.ActivationFunctionType
OP = mybir.AluOpType

# problem shape (hardcoded per spec)
B, Q, K, F, C = 16, 2048, 2048, 128, 2
NCORES = 8
BL = B // NCORES  # batches per core
P = 128
QT = Q // P  # q tiles per batch
KC = K // P  # k chunks (for PV matmul)
NB = 512  # one PSUM bank of fp32
HALF = 1024  # psum tile free size (2 banks)
PH = 8  # q-tiles per exp/sqrt phase group
SCALE = float(F) ** -0.5
EPS_D2 = 1e-5  # fp32 rounding guard so sqrt input never goes negative

_CACHE = {}


def build_program(ablate=None):
    nc = bacc.Bacc("TRN2", target_bir_lowering=False, debug=False)

    qT_d = nc.dram_tensor("qT", [BL, F, Q], DT.float32, kind="ExternalInput")
    kT_d = nc.dram_tensor("kT", [BL, F, K], DT.float32, kind="ExternalInput")
    # v pre-chunked on host: v_d[b, p, t, f] = v[b, t*128+p, f]
    v_d = nc.dram_tensor("v", [BL, P, KC, F], DT.float32, kind="ExternalInput")
    # coords transposed + flattened: [x row | y row] on one partition
    cqT_d = nc.dram_tensor("cqT", [BL, 1, C * Q], DT.float32, kind="ExternalInput")
    ckT_d = nc.dram_tensor("ckT", [BL, 1, C * K], DT.float32, kind="ExternalInput")
    cqn_d = nc.dram_tensor("cqn", [BL, Q, C], DT.float32, kind="ExternalInput")
    tn_d = nc.dram_tensor("tn", [BL, Q, K], DT.uint8, kind="ExternalInput")
    an_d = nc.dram_tensor("an", [BL, Q, K], DT.uint8, kind="ExternalInput")
    bs_d = nc.dram_tensor("bs", [P, 1], DT.float32, kind="ExternalInput")
    rm_d = nc.dram_tensor("rm", [P, 1], DT.float32, kind="ExternalInput")
    ones16_d = nc.dram_tensor("ones16", [1, Q], DT.float16, kind="ExternalInput")
    out_d = nc.dram_tensor("out", [BL, Q, F], DT.float32, kind="ExternalOutput")

    with tile.TileContext(nc) as tc:
        with (
            tc.tile_pool(name="consts", bufs=1) as consts,
            tc.tile_pool(name="stage", bufs=2) as stage,
            tc.tile_pool(name="binp", bufs=2) as binp,  # per-batch fp16 inputs
            tc.tile_pool(name="augp", bufs=1) as augp,
            tc.tile_pool(name="rowp", bufs=2) as rowp,
            tc.tile_pool(name="Ep", bufs=PH + 3) as Ep,
            tc.tile_pool(name="maskp", bufs=5) as maskp,
            tc.tile_pool(name="zsp", bufs=2) as zsp,
            tc.tile_pool(name="ptp", bufs=4) as ptp,
            tc.tile_pool(name="smallp", bufs=PH + 4) as smallp,
            tc.tile_pool(name="outp", bufs=3) as outp,
            tc.tile_pool(name="psp", bufs=3, space="PSUM") as psp,
            tc.tile_pool(name="attps", bufs=2, space="PSUM") as attps,
        ):
            # ---------- per-core constants
            bs_sb = consts.tile([P, 1], DT.float32)
            nc.sync.dma_start(bs_sb, bs_d[:])
            rm_sb = consts.tile([P, 1], DT.float32)
            nc.sync.dma_start(rm_sb, rm_d[:])
            zeros16 = consts.tile([P, K], DT.float16)
            nc.vector.memset(zeros16, 0.0)
            c2_sb = consts.tile([P, 1], DT.float32)
            # c2 = (bs / rm)^2
            nc.vector.reciprocal(c2_sb, rm_sb[:])
            nc.vector.tensor_tensor(c2_sb, c2_sb[:], bs_sb[:], OP.mult)
            nc.vector.tensor_tensor(c2_sb, c2_sb[:], c2_sb[:], OP.mult)

            # PV work queue, drained a few chunk-matmuls at a time between
            # other PE work so xbar-transpose latency never stalls the PE
            # stream and PV never bunches up.
            pv_q = []  # dicts: b, t, pt_sb, r_sb, att_ps, next_c, age
            v_sbs = {}
            slot_serial = [0]

            def drain_pv(budget, after=None, min_age=2):
                first = True
                while budget > 0 and pv_q:
                    job = pv_q[0]
                    if slot_serial[0] - job["age"] < min_age and min_age > 0:
                        break
                    if job["att_ps"] is None:
                        job["att_ps"] = attps.tile(
                            [P, F], DT.float32, tag="att", name="att_ps"
                        )
                    c = job["next_c"]
                    mm = nc.tensor.matmul(
                        job["att_ps"][:],
                        job["pt_sb"][:, c * P : (c + 1) * P],
                        v_sbs[job["b"]][:, c, :],
                        start=(c == 0),
                        stop=(c == KC - 1),
                    )
                    if first and after is not None:
                        add_dep_helper(
                            mm.ins, after.ins, sync=False,
                            reason="PE order: PV chunks after dist/S matmuls",
                        )
                        first = False
                    job["next_c"] += 1
                    budget -= 1
                    if job["next_c"] == KC:
                        o_sb = outp.tile([P, F], DT.float32, tag="o", name="o_sb")
                        nc.vector.tensor_scalar(
                            o_sb, job["att_ps"][:], job["r_sb"][:], None, OP.mult
                        )
                        nc.sync.dma_start(
                            out_d[job["b"], job["t"] * P : (job["t"] + 1) * P, :], o_sb
                        )
                        pv_q.pop(0)

            for b in range(BL):
                # ---------- per-batch setup
                qT_st = stage.tile([P, Q], DT.float32, tag="qkstage")
                nc.gpsimd.dma_start(qT_st, qT_d[b])
                qT_sb = binp.tile([P, Q], DT.float16, tag="qTb")
                nc.vector.tensor_copy(qT_sb, qT_st[:])

                kT_st = stage.tile([P, K], DT.float32, tag="qkstage")
                nc.gpsimd.dma_start(kT_st, kT_d[b])
                kT_sb = binp.tile([P, K], DT.float16, tag="kTb")
                nc.vector.tensor_copy(kT_sb, kT_st[:])

                v_st = stage.tile([P, KC, F], DT.float32, tag="qkstage")
                nc.gpsimd.dma_start(v_st, v_d[b])
                v_sb = binp.tile([P, KC, F], DT.float16, tag="vb")
                nc.vector.tensor_copy(v_sb, v_st[:])
                v_sbs[b] = v_sb

                # per-q |cq|^2 column from fp16-rounded coords (fp32 result)
                cqn_st = stage.tile([P, QT, C], DT.float32, tag="cqn_st")
                nc.gpsimd.dma_start(cqn_st, cqn_d[b].rearrange("(t p) c -> p t c", p=P))
                cqn16 = stage.tile([P, QT, C], DT.float16, tag="cqn16")
                nc.vector.tensor_copy(cqn16, cqn_st[:])
                sqn = stage.tile([P, QT, C], DT.float32, tag="sqn")
                nc.vector.tensor_tensor(sqn, cqn16[:], cqn16[:], OP.mult)
                nq_col = binp.tile([P, QT], DT.float32, tag="nq_col")
                nc.vector.tensor_reduce(nq_col, sqn[:], axis=mybir.AxisListType.X, op=OP.add)

                # ---------- augmented distance matrices (all fp16-exact)
                # Aq rows: [1, 1, xq, yq]   Ak rows: [nk_hi, nk_lo, -2xk, -2yk]
                Aq_sb = augp.tile([4, Q], DT.float16, tag="Aq")
                Ak_sb = augp.tile([4, K], DT.float16, tag="Ak")
                # q side: fp16 coords
                row_q = rowp.tile([1, C * Q], DT.float32, tag="scr32", bufs=1)
                nc.gpsimd.dma_start(row_q, cqT_d[b])
                xyq16 = rowp.tile([1, C * Q], DT.float16, tag="scr16")
                nc.vector.tensor_copy(xyq16, row_q[:])
                nc.sync.dma_start(Aq_sb[0:1, :], ones16_d[:])
                nc.sync.dma_start(Aq_sb[1:2, :], ones16_d[:])
                nc.sync.dma_start(Aq_sb[2:3, :], xyq16[:, 0:Q])
                nc.sync.dma_start(Aq_sb[3:4, :], xyq16[:, Q : 2 * Q])
                # k side: -2*fp16 coords (exact doubling) + double-fp16 norms
                row_k = rowp.tile([1, C * K], DT.float32, tag="scr32", bufs=1)
                nc.gpsimd.dma_start(row_k, ckT_d[b])
                xyk16 = rowp.tile([1, C * K], DT.float16, tag="scr16")
                nc.vector.tensor_copy(xyk16, row_k[:])
                m2k16 = rowp.tile([1, C * K], DT.float16, tag="scr16")
                nc.vector.tensor_scalar(m2k16, xyk16[:], -2.0, None, OP.mult)
                sqk = rowp.tile([1, C * K], DT.float32, tag="scr32", bufs=1)
                nc.vector.tensor_tensor(sqk, xyk16[:], xyk16[:], OP.mult)
                nk32 = rowp.tile([1, K], DT.float32, tag="nk32", bufs=1)
                nc.vector.tensor_tensor(nk32, sqk[:, 0:K], sqk[:, K : 2 * K], OP.add)
                nk_hi = rowp.tile([1, K], DT.float16, tag="scr16")
                nc.vector.tensor_copy(nk_hi, nk32[:])
                nk_lo = rowp.tile([1, K], DT.float16, tag="scr16")
                nc.vector.tensor_tensor(nk_lo, nk32[:], nk_hi[:], OP.subtract)
                nc.sync.dma_start(Ak_sb[0:1, :], nk_hi[:])
                nc.sync.dma_start(Ak_sb[1:2, :], nk_lo[:])
                nc.sync.dma_start(Ak_sb[2:3, :], m2k16[:, 0:K])
                nc.sync.dma_start(Ak_sb[3:4, :], m2k16[:, K : 2 * K])

                for g in range(QT // PH):
                    # ------ phase A: S matmuls + exp (exp table set)
                    E_tiles, u2_tiles, eb_tiles, r_tiles = [], [], [], []
                    for t in range(g * PH, (g + 1) * PH):
                        E_sb = Ep.tile([P, K], DT.float16, tag="E")
                        s01 = smallp.tile([P, 2], DT.float32, tag="s01")
                        for h in range(2):
                            ps = psp.tile([P, HALF], DT.float32, tag="ps")
                            for j in range(2):
                                col = 2 * h + j
                                last_s = nc.tensor.matmul(
                                    ps[:, j * NB : (j + 1) * NB],
                                    qT_sb[:, t * P : (t + 1) * P],
                                    kT_sb[:, col * NB : (col + 1) * NB],
                                    start=True,
                                    stop=True,
                                )
                            nc.scalar.activation(
                                E_sb[:, h * HALF : (h + 1) * HALF],
                                ps[:],
                                AF.Exp,
                                bias=0.0,
                                scale=SCALE,
                                accum_out=s01[:, h : h + 1],
                            )
                        s_sb = smallp.tile([P, 1], DT.float32, tag="s")
                        nc.vector.tensor_tensor(s_sb, s01[:, 0:1], s01[:, 1:2], OP.add)
                        u2_sb = smallp.tile([P, 1], DT.float32, tag="u2")
                        nc.vector.tensor_tensor(u2_sb, s_sb[:], s_sb[:], OP.mult)
                        nc.vector.tensor_tensor(u2_sb, u2_sb[:], c2_sb[:], OP.mult)
                        # sqrt bias = u2 * (nq + eps)
                        eb_sb = smallp.tile([P, 1], DT.float32, tag="eb")
                        nc.vector.scalar_tensor_tensor(
                            eb_sb, nq_col[:, t : t + 1], EPS_D2, u2_sb[:], OP.add, OP.mult
                        )
                        r_sb = smallp.tile([P, 1], DT.float32, tag="r")
                        nc.vector.reciprocal(r_sb, s_sb[:])
                        slot_serial[0] += 1
                        drain_pv(8, after=last_s)
                        E_tiles.append(E_sb)
                        u2_tiles.append(u2_sb)
                        eb_tiles.append(eb_sb)
                        r_tiles.append(r_sb)

                    # ------ phase B: dist matmuls + sqrt + combine (sqrt set)
                    for i, t in enumerate(range(g * PH, (g + 1) * PH)):
                        E_sb = E_tiles[i]
                        zs_sb = zsp.tile([P, K], DT.float16, tag="zs")
                        last_d2 = None
                        for h in range(2 if ablate != "nodist" else 0):
                            psd = psp.tile([P, HALF], DT.float32, tag="ps")
                            for j in range(2):
                                col = 2 * h + j
                                last_d2 = nc.tensor.matmul(
                                    psd[:, j * NB : (j + 1) * NB],
                                    Aq_sb[:, t * P : (t + 1) * P],
                                    Ak_sb[:, col * NB : (col + 1) * NB],
                                    start=True,
                                    stop=True,
                                )
                            # zs = sqrt(u2*d2 + u2*(nq+eps)) = s*c*dist
                            nc.scalar.activation(
                                zs_sb[:, h * HALF : (h + 1) * HALF],
                                psd[:],
                                AF.Sqrt,
                                bias=eb_tiles[i][:],
                                scale=u2_tiles[i][:],
                            )
                        if ablate != "nomask":
                            an_sb = maskp.tile([P, K], DT.uint8, tag="an")
                            nc.gpsimd.dma_start(an_sb, an_d[b, t * P : (t + 1) * P, :])
                            tn_sb = maskp.tile([P, K], DT.uint8, tag="tn")
                            nc.gpsimd.dma_start(tn_sb, tn_d[b, t * P : (t + 1) * P, :])

                        # P = where(attn, 0, E - where(alibi, 0, zs))  (in place)
                        if ablate == "nomask":
                            if ablate != "nodist":
                                nc.vector.tensor_tensor(E_sb, E_sb[:], zs_sb[:], OP.subtract)
                        elif ablate == "nodist":
                            nc.vector.copy_predicated(E_sb, tn_sb[:], zeros16[:])
                        else:
                            nc.vector.copy_predicated(zs_sb, an_sb[:], zeros16[:])
                            nc.vector.tensor_tensor(E_sb, E_sb[:], zs_sb[:], OP.subtract)
                            nc.vector.copy_predicated(E_sb, tn_sb[:], zeros16[:])

                        if ablate == "nopv":
                            o_sb = outp.tile([P, F], DT.float32, tag="o")
                            nc.vector.tensor_scalar(
                                o_sb, E_sb[:, 0:F], r_tiles[i][:], None, OP.mult
                            )
                            nc.sync.dma_start(out_d[b, t * P : (t + 1) * P, :], o_sb)
                            slot_serial[0] += 1
                            continue
                        # transpose P -> PT chunks: one xbar transpose does all
                        # 16 chunks (out[p, t, f] = P[f, t*128+p])
                        pt_sb = ptp.tile([P, K], DT.float16, tag="pt")
                        nc.sync.dma_start_transpose(
                            pt_sb[:].rearrange("p (t f) -> p t f", t=KC),
                            E_sb[:],
                        )
                        pv_q.append(dict(
                            b=b, t=t, pt_sb=pt_sb, r_sb=r_tiles[i],
                            att_ps=None, next_c=0, age=slot_serial[0],
                        ))
                        slot_serial[0] += 1
                        drain_pv(8, after=last_d2)

            drain_pv(10**9, min_age=0)

    nc.compile()
    return nc


def _prep_in_maps(q, k, v, coords_q, coords_k, attn_mask, alibi_mask, bias_scale, running_mean):
    q = np.asarray(q, dtype=np.float32)
    k = np.asarray(k, dtype=np.float32)
    v = np.asarray(v, dtype=np.float32)
    coords_q = np.asarray(coords_q, dtype=np.float32)
    coords_k = np.asarray(coords_k, dtype=np.float32)
    # raw masks as u8 predicates for copy_predicated
    tn = np.asarray(attn_mask, dtype=bool).astype(np.uint8)
    an = np.asarray(alibi_mask, dtype=bool).astype(np.uint8)
    bs = np.broadcast_to(np.asarray(bias_scale, np.float32).reshape(1, 1), (P, 1)).copy()
    rm = np.broadcast_to(np.asarray(running_mean, np.float32).reshape(1, 1), (P, 1)).copy()
    ones16 = np.ones((1, Q), dtype=np.float16)

    in_maps = []
    for i in range(NCORES):
        sl = slice(i * BL, (i + 1) * BL)
        in_maps.append(
            dict(
                qT=np.ascontiguousarray(q[sl].transpose(0, 2, 1)),
                kT=np.ascontiguousarray(k[sl].transpose(0, 2, 1)),
                v=np.ascontiguousarray(
                    v[sl].reshape(BL, KC, P, F).transpose(0, 2, 1, 3)
                ),
                cqT=np.ascontiguousarray(coords_q[sl].transpose(0, 2, 1)).reshape(BL, 1, C * Q),
                ckT=np.ascontiguousarray(coords_k[sl].transpose(0, 2, 1)).reshape(BL, 1, C * K),
                cqn=np.ascontiguousarray(coords_q[sl]),
                tn=np.ascontiguousarray(tn[sl]),
                an=np.ascontiguousarray(an[sl]),
                bs=bs,
                rm=rm,
                ones16=ones16,
            )
        )
    return in_maps


def kernel(q, k, v, coords_q, coords_k, attn_mask, alibi_mask, bias_scale, running_mean):
    from concourse.bass_utils import run_bass_kernel_spmd

    if "nc" not in _CACHE:
        _CACHE["nc"] = build_program()
    nc = _CACHE["nc"]

    in_maps = _prep_in_maps(
        q, k, v, coords_q, coords_k, attn_mask, alibi_mask, bias_scale, running_mean
    )
    res = run_bass_kernel_spmd(nc, in_maps, core_ids=list(range(NCORES)))
    _CACHE["last_results"] = res
    out = np.concatenate([res.results[i]["out"] for i in range(NCORES)], axis=0)
    return out.astype(np.float32)
